# revision 14
# baseline (speedup 1.0000x reference)
"""Trainium2 Bass kernel: AnaphoricityScorer (wl-coref pair FFNN scorer).

Data-parallel over the 512-row mention batch across 8 NeuronCores (64 rows
per core).  Per core (3200 pairs):

  1. The gather b = all_mentions[top_indices] is done with the GPSIMD
     dma_gather(transpose=True) custom DMA, which lands the gathered fp16
     embeddings TRANSPOSED in SBUF: out[p, c, n] = table[idx_n, 128c+p].
     That puts the contraction dim (embedding) on partitions, which is
     exactly the layout the TensorEngine needs for the moving operand.
  2. s = a * b (similarity) is one DVE multiply per block against a
     pre-broadcast a^T tile (built on-device from mT with one DVE copy).
  3. hT[h, pair] = W1b^T b + W1s^T s + W1p^T pw + (a@W1a broadcast) is
     accumulated in PSUM via fp16 matmuls with the W1 chunks as stationary
     operands (the a-term enters via a one-hot moving operand against the
     on-device-computed ma = mentions@W1a).
  4. One ScalarEngine activation applies  leaky_relu(hT + b1)  straight out
     of PSUM into SBUF (fp16).
  5. Layer 2 uses hrelu slices as the stationary operand: out[pair, 1] =
     hrelu_slice^T @ W_out; + b_out (activation bias) + rough (DVE add).

Pair order is "antecedent-major" (p' = j*64 + i) so the a-broadcast is a
clean 64-wide repeat; all permutation/layout work is host-side.

Block sizes are uneven: a small first block shortens the pipeline lead-in
(descriptor generation is ~9ns/desc on the Q7), a small last block
shortens the tail.  Gathers spread across the 4 SWDGE queues so their
descriptor generations run on different Q7 core pairs concurrently.
Constant inputs ride in 3 packed DMAs to keep descriptor count low.
"""

import numpy as np

N_MENTIONS = 10000
BATCH = 512
N_ANTS = 50
EMB = 1024
PW = 64
HID = 128
N_CORES = 8
R = BATCH // N_CORES            # 64 rows per core
NPAIR = R * N_ANTS              # 3200 pairs per core
NCH = EMB // 128                # 8 embedding chunks
EPS = 1e-7
LEAKY = 0.01

SIZES = [128, 768, 768, 768, 640, 128]      # pairs per pipeline block
OFFS = np.cumsum([0] + SIZES).tolist()
NB = len(SIZES)
BLKMAX = max(SIZES)
assert OFFS[-1] == NPAIR and all(s % 128 == 0 for s in SIZES)

# wcat column layout (fp16, 128 partitions)
WB0, WS0, WA0 = 0, NCH * HID, 2 * NCH * HID
MT0 = 3 * NCH * HID
WO0 = MT0 + NCH * R
WCAT = WO0 + 1
# pcat column layout (fp16, 64 partitions)
PPW0, PE0, PW1P0 = 0, NPAIR, 2 * NPAIR
PCAT = 2 * NPAIR + HID
# fcat column layout (fp32, 128 partitions)
FB10, FBO0, FRG0 = 0, 1, 2
FCAT = 2 + NPAIR // 128

_CACHE = {}


def _build():
    """Build + compile the (SPMD, per-core identical) Bass program."""
    if "nc" in _CACHE:
        return _CACHE["nc"]
    from concourse import bacc, mybir
    import concourse.tile as tile

    f16, f32, i16 = mybir.dt.float16, mybir.dt.float32, mybir.dt.int16
    AF = mybir.ActivationFunctionType
    nc = bacc.Bacc(num_swdge_queues=4)

    def inp(name, shape, dtype):
        return nc.declare_dram_parameter(name, list(shape), dtype, isOutput=False)

    table = inp("table", [N_MENTIONS, EMB], f16)
    idx = inp("idx", [128, NPAIR // 16], i16)
    wcat = inp("wcat", [128, WCAT], f16)
    pcat = inp("pcat", [PW, PCAT], f16)
    fcat = inp("fcat", [128, FCAT], f32)
    out = nc.declare_dram_parameter("out", [128, NPAIR // 128], f32, isOutput=True)

    with tile.TileContext(nc) as tc:
        with (
            tc.tile_pool(name="const", bufs=1) as cp,
            tc.tile_pool(name="bt", bufs=NB) as btp,
            tc.tile_pool(name="st", bufs=3) as stp,
            tc.tile_pool(name="hr", bufs=2) as hrp,
            tc.tile_pool(name="sm", bufs=2) as smp,
            tc.tile_pool(name="psH", bufs=2, space="PSUM") as psH,
            tc.tile_pool(name="psS", bufs=2, space="PSUM") as psS,
            tc.tile_pool(name="psM", bufs=1, space="PSUM") as psM,
        ):
            idx_sb = cp.tile([128, NPAIR // 16], i16, tag="idx")
            nc.sync.dma_start(out=idx_sb[:], in_=idx[:])

            wcat_sb = cp.tile([128, WCAT], f16, tag="wcat")
            nc.scalar.dma_start(out=wcat_sb[:], in_=wcat[:])
            pcat_sb = cp.tile([PW, PCAT], f16, tag="pcat")
            nc.scalar.dma_start(out=pcat_sb[:], in_=pcat[:])
            fcat_sb = cp.tile([128, FCAT], f32, tag="fcat")
            nc.scalar.dma_start(out=fcat_sb[:], in_=fcat[:])

            def wb_c(c):
                return wcat_sb[:, WB0 + c * HID:WB0 + (c + 1) * HID]

            def ws_c(c):
                return wcat_sb[:, WS0 + c * HID:WS0 + (c + 1) * HID]

            def wa_c(c):
                return wcat_sb[:, WA0 + c * HID:WA0 + (c + 1) * HID]

            def mT_c(c):
                return wcat_sb[:, MT0 + c * R:MT0 + (c + 1) * R]

            wout_sb = wcat_sb[:, WO0:WO0 + 1]
            w1p_sb = pcat_sb[:, PW1P0:PW1P0 + HID]
            b1_sb = fcat_sb[:, FB10:FB10 + 1]
            bout_sb = fcat_sb[:, FBO0:FBO0 + 1]

            # Kick off all gathers as early as possible (desc-gen runs on a
            # Q7 core pair selected by queue_num, so spreading queues lets
            # up to 4 descriptor generations run concurrently).
            bts = []
            for b in range(NB):
                L, o = SIZES[b], OFFS[b]
                bt = btp.tile([128, NCH * L], f16, tag="bt")
                nc.gpsimd.dma_gather(
                    out_ap=bt[:].rearrange("p (c n) -> p c n", c=NCH),
                    in_ap=table[:],
                    idxs_ap=idx_sb[:, o // 16:(o + L) // 16],
                    num_idxs=L,
                    num_idxs_reg=L,
                    elem_size=EMB,
                    transpose=True,
                    queue_num=b % 4,
                )
                bts.append(bt)

            # aT = per-block a^T broadcast (j-repeat of mT) built on-device.
            aT_sb = cp.tile([128, NCH * BLKMAX], f16, tag="aT")
            nc.vector.tensor_copy(
                aT_sb[:].rearrange("p (c j i) -> p c j i", c=NCH, j=BLKMAX // R),
                wcat_sb[:, MT0:MT0 + NCH * R]
                .rearrange("p (c i) -> p c i", c=NCH)[:, :, None, :]
                .broadcast_to([128, NCH, BLKMAX // R, R]),
            )

            scores_sb = cp.tile([128, NPAIR // 128], f32, tag="scores")

            # ma = mentions_shard @ W1a  -> [R, HID]
            ma_ps = psM.tile([R, HID], f32)
            for c in range(NCH):
                nc.tensor.matmul(
                    ma_ps[:],
                    lhsT=mT_c(c),
                    rhs=wa_c(c),
                    start=(c == 0),
                    stop=(c == NCH - 1),
                )
            ma_sb = cp.tile([R, HID], f16, tag="ma")
            nc.scalar.activation(ma_sb[:], ma_ps[:], AF.Copy)

            for b in range(NB):
                L, o = SIZES[b], OFFS[b]
                bt = bts[b]
                st = stp.tile([128, NCH * L], f16, tag="st")
                nc.vector.tensor_mul(
                    st[:].rearrange("p (c n) -> p c n", c=NCH),
                    bt[:].rearrange("p (c n) -> p c n", c=NCH),
                    aT_sb[:].rearrange("p (c n) -> p c n", c=NCH)[:, :, :L],
                )

                hT = psH.tile([128, L], f32, tag="hT")
                nsub = [(0, min(512, L))] + ([(512, L)] if L > 512 else [])
                for lo, hi in nsub:
                    for c in range(NCH):
                        nc.tensor.matmul(
                            hT[:, lo:hi],
                            lhsT=wb_c(c),
                            rhs=bt[:, c * L + lo:c * L + hi],
                            start=(c == 0),
                            stop=False,
                        )
                    for c in range(NCH):
                        nc.tensor.matmul(
                            hT[:, lo:hi],
                            lhsT=ws_c(c),
                            rhs=st[:, c * L + lo:c * L + hi],
                            start=False,
                            stop=False,
                        )
                    nc.tensor.matmul(
                        hT[:, lo:hi],
                        lhsT=w1p_sb,
                        rhs=pcat_sb[:, PPW0 + o + lo:PPW0 + o + hi],
                        start=False,
                        stop=False,
                    )
                    nc.tensor.matmul(
                        hT[:, lo:hi],
                        lhsT=ma_sb[:],
                        rhs=pcat_sb[:, PE0 + o + lo:PE0 + o + hi],
                        start=False,
                        stop=True,
                    )

                hr = hrp.tile([128, L], f16, tag="hr")
                nc.scalar.activation(
                    hr[:], hT[:], AF.Lrelu, bias=b1_sb, scale=1.0, alpha=LEAKY
                )

                ng = L // 128
                sc = psS.tile([128, ng], f32, tag="sc")
                for g in range(ng):
                    nc.tensor.matmul(
                        sc[:, g:g + 1],
                        lhsT=hr[:, g * 128:(g + 1) * 128],
                        rhs=wout_sb,
                        start=True,
                        stop=True,
                    )
                tmp = smp.tile([128, ng], f32, tag="tmp")
                nc.scalar.activation(
                    tmp[:], sc[:], AF.Identity, bias=bout_sb, scale=1.0
                )
                nc.vector.tensor_add(
                    scores_sb[:, o // 128:(o + L) // 128],
                    tmp[:],
                    fcat_sb[:, FRG0 + o // 128:FRG0 + (o + L) // 128],
                )

            nc.sync.dma_start(out=out[:], in_=scores_sb[:])

    nc.compile()
    _CACHE["nc"] = nc
    return nc


def _chunkT(w):
    # [1024, 128] -> [128, 8*128] fp16: column c*128+h holds W[c*128+r, h]
    return np.ascontiguousarray(
        w.reshape(NCH, 128, HID).transpose(1, 0, 2).reshape(128, NCH * HID)
    ).astype(np.float16)


def _host_shared(inputs):
    table = np.asarray(inputs["all_mentions"], np.float32).astype(np.float16)
    W1 = np.asarray(inputs["W1"], np.float32)
    w1a, w1b, w1s, w1p = W1[:1024], W1[1024:2048], W1[2048:3072], W1[3072:]
    return {
        "table": np.ascontiguousarray(table),
        "_wb": _chunkT(w1b),
        "_ws": _chunkT(w1s),
        "_wa": _chunkT(w1a),
        "_w1p": np.ascontiguousarray(w1p).astype(np.float16),
        "_e64": np.ascontiguousarray(
            np.tile(np.eye(R, dtype=np.float16), (1, N_ANTS))
        ),
        "_wout": np.asarray(inputs["W_out"], np.float32).astype(np.float16),
        "_b1c": np.asarray(inputs["b1"], np.float32).reshape(HID, 1),
        "_boutc": np.full(
            (128, 1), np.asarray(inputs["b_out"], np.float32).reshape(())
        ),
    }


def _host_core(inputs, shared, c):
    sl = slice(c * R, (c + 1) * R)
    m = np.asarray(inputs["mentions_batch"], np.float32)[sl]          # [64, 1024]
    pw = np.asarray(inputs["pw_batch"], np.float32)[sl]               # [64, 50, 64]
    idx = np.asarray(inputs["top_indices_batch"])[sl].astype(np.int64)
    rough = np.asarray(inputs["top_rough_scores_batch"], np.float32)[sl]

    idx_perm = idx.T.reshape(NPAIR).astype(np.int16)                  # p' = j*R + i
    idx16 = np.concatenate(
        [
            np.tile(
                idx_perm[OFFS[b]:OFFS[b + 1]].reshape(SIZES[b] // 16, 16).T,
                (8, 1),
            )
            for b in range(NB)
        ],
        axis=1,
    )                                                                 # [128, 200]

    mT = m.reshape(R, NCH, 128).transpose(2, 1, 0).reshape(128, NCH * R)
    pwT = pw.transpose(1, 0, 2).reshape(NPAIR, PW).T                  # [64, 3200]
    rough_pp = rough.T.reshape(NPAIR).reshape(NPAIR // 128, 128).T    # [128, 25]

    wcat = np.concatenate(
        [shared["_wb"], shared["_ws"], shared["_wa"], mT.astype(np.float16),
         shared["_wout"]],
        axis=1,
    )
    pcat = np.concatenate(
        [pwT.astype(np.float16), shared["_e64"], shared["_w1p"]], axis=1
    )
    fcat = np.concatenate(
        [shared["_b1c"], shared["_boutc"], rough_pp], axis=1
    ).astype(np.float32)

    return {
        "idx": np.ascontiguousarray(idx16),
        "wcat": np.ascontiguousarray(wcat),
        "pcat": np.ascontiguousarray(pcat),
        "fcat": np.ascontiguousarray(fcat),
    }


def make_in_maps(inputs):
    shared = _host_shared(inputs)
    table = shared["table"]
    return [
        {"table": table, **_host_core(inputs, shared, c)} for c in range(N_CORES)
    ]


def assemble_output(inputs, results):
    """results: list of per-core dicts with 'out' [128, 25] -> [512, 51] f32."""
    scores = np.empty((BATCH, N_ANTS), np.float32)
    for c in range(N_CORES):
        out_flat = np.asarray(results[c]["out"], np.float32).T.reshape(NPAIR)
        scores[c * R:(c + 1) * R] = out_flat.reshape(N_ANTS, R).T
    dummy = np.full((BATCH, 1), EPS, np.float32)
    return np.concatenate([dummy, scores], axis=1)


def run(inputs, trace=False, **kwargs):
    """Compile (cached), run on 8 cores, return (output, BassKernelResults)."""
    from concourse.bass_utils import run_bass_kernel_spmd

    nc = _build()
    in_maps = make_in_maps(inputs)
    res = run_bass_kernel_spmd(
        nc, in_maps, core_ids=list(range(N_CORES)), trace=trace, **kwargs
    )
    return assemble_output(inputs, res.results), res


def kernel(**inputs) -> np.ndarray:
    out, _ = run(inputs, trace=False)
    return out


# revision 17
# speedup vs baseline: 1.0740x; 1.0740x over previous
"""Trainium2 Bass kernel: AnaphoricityScorer (wl-coref pair FFNN scorer).

Data-parallel over the 512-row mention batch across 8 NeuronCores (64 rows
per core).  Per core (3200 pairs):

  1. The gather b = all_mentions[top_indices] is done with the GPSIMD
     dma_gather(transpose=True) custom DMA, which lands the gathered fp16
     embeddings TRANSPOSED in SBUF: out[p, c, n] = table[idx_n, 128c+p].
     That puts the contraction dim (embedding) on partitions, which is
     exactly the layout the TensorEngine needs for the moving operand.
  2. s = a * b (similarity) is one DVE multiply per block against a
     pre-broadcast a^T tile (built on-device from mT with one DVE copy).
  3. hT[h, pair] = W1b^T b + W1s^T s + W1p^T pw + (a@W1a broadcast) is
     accumulated in PSUM via fp16 matmuls with the W1 chunks as stationary
     operands (the a-term enters via a one-hot moving operand against the
     on-device-computed ma = mentions@W1a).
  4. One ScalarEngine activation applies  leaky_relu(hT + b1)  straight out
     of PSUM into SBUF (fp16).
  5. Layer 2 uses hrelu slices as the stationary operand: out[pair, 1] =
     hrelu_slice^T @ W_out; + b_out (activation bias) + rough (DVE add).

Pair order is "antecedent-major" (p' = j*64 + i) so the a-broadcast is a
clean 64-wide repeat; all permutation/layout work is host-side.

Block sizes are uneven: a small first block shortens the pipeline lead-in
(descriptor generation is ~9ns/desc on the Q7), a small last block
shortens the tail.  Gathers spread across the 4 SWDGE queues so their
descriptor generations run on different Q7 core pairs concurrently.
Constant inputs ride in 3 packed DMAs to keep descriptor count low.
"""

import numpy as np

N_MENTIONS = 10000
BATCH = 512
N_ANTS = 50
EMB = 1024
PW = 64
HID = 128
N_CORES = 8
R = BATCH // N_CORES            # 64 rows per core
NPAIR = R * N_ANTS              # 3200 pairs per core
NCH = EMB // 128                # 8 embedding chunks
EPS = 1e-7
LEAKY = 0.01
N_WARM = 48                     # PE warm-up matmuls (HAM unthrottle)

SIZES = [128, 640, 640, 640, 640, 512]      # pairs per pipeline block
OFFS = np.cumsum([0] + SIZES).tolist()
NB = len(SIZES)
BLKMAX = max(SIZES)
assert OFFS[-1] == NPAIR and all(s % 128 == 0 for s in SIZES)

# wcat column layout (fp16, 128 partitions)
WB0, WS0, WA0 = 0, NCH * HID, 2 * NCH * HID
MT0 = 3 * NCH * HID
WO0 = MT0 + NCH * R
WCAT = WO0 + 1
# pcat column layout (fp16, 64 partitions)
PPW0, PE0, PW1P0 = 0, NPAIR, 2 * NPAIR
PCAT = 2 * NPAIR + HID
# fcat column layout (fp32, 128 partitions)
FB10, FBO0, FRG0 = 0, 1, 2
FCAT = 2 + NPAIR // 128

_CACHE = {}


def _build():
    """Build + compile the (SPMD, per-core identical) Bass program."""
    if "nc" in _CACHE:
        return _CACHE["nc"]
    from concourse import bacc, mybir
    import concourse.tile as tile

    f16, f32, i16 = mybir.dt.float16, mybir.dt.float32, mybir.dt.int16
    AF = mybir.ActivationFunctionType
    nc = bacc.Bacc(num_swdge_queues=4)

    def inp(name, shape, dtype):
        return nc.declare_dram_parameter(name, list(shape), dtype, isOutput=False)

    table = inp("table", [N_MENTIONS, EMB], f16)
    idx = inp("idx", [128, NPAIR // 16], i16)
    wcat = inp("wcat", [128, WCAT], f16)
    pcat = inp("pcat", [PW, PCAT], f16)
    fcat = inp("fcat", [128, FCAT], f32)
    out = nc.declare_dram_parameter("out", [128, NPAIR // 128], f32, isOutput=True)

    with tile.TileContext(nc) as tc:
        with (
            tc.tile_pool(name="const", bufs=1) as cp,
            tc.tile_pool(name="bt", bufs=NB) as btp,
            tc.tile_pool(name="st", bufs=3) as stp,
            tc.tile_pool(name="hr", bufs=2) as hrp,
            tc.tile_pool(name="sm", bufs=2) as smp,
            tc.tile_pool(name="psH", bufs=2, space="PSUM") as psH,
            tc.tile_pool(name="psS", bufs=2, space="PSUM") as psS,
            tc.tile_pool(name="psM", bufs=1, space="PSUM") as psM,
        ):
            # --- warm-up: PE spin + dma_gather first-use probe -----------
            # The PE HAM clock gate needs ~3.4us of sustained matmul work to
            # unthrottle 1.2 -> 2.4 GHz; dependency-free dummy matmuls keep
            # it busy while inputs load, so the real matmuls run warm.
            warm_a = cp.tile([128, 128], f16, tag="warm_a")
            warm_b = cp.tile([128, 512], f16, tag="warm_b")
            nc.gpsimd.memset(warm_a[:], 0)
            nc.gpsimd.memset(warm_b[:], 0)
            warm_ps = psM.tile([128, 512], f32, tag="warm_ps")
            for _ in range(N_WARM):
                nc.tensor.matmul(
                    warm_ps[:], lhsT=warm_a[:], rhs=warm_b[:], start=True, stop=True
                )
            # The first dma_gather execution pays a ~10us one-time cost
            # (ucode library page-in); a dependency-free dummy gather (all
            # indices 0 via memset) absorbs it while inputs stream in.
            scrap_idx = cp.tile([128, 8], i16, tag="scrap_idx")
            nc.gpsimd.memset(scrap_idx[:], 0)
            scrap_bt = cp.tile([128, NCH * 128], f16, tag="scrap_bt")
            nc.gpsimd.dma_gather(
                out_ap=scrap_bt[:].rearrange("p (c n) -> p c n", c=NCH),
                in_ap=table[:],
                idxs_ap=scrap_idx[:],
                num_idxs=128,
                num_idxs_reg=128,
                elem_size=EMB,
                transpose=True,
                queue_num=3,
            )

            idx_sb = cp.tile([128, NPAIR // 16], i16, tag="idx")
            nc.sync.dma_start(out=idx_sb[:], in_=idx[:])

            wcat_sb = cp.tile([128, WCAT], f16, tag="wcat")
            nc.scalar.dma_start(out=wcat_sb[:], in_=wcat[:])
            pcat_sb = cp.tile([PW, PCAT], f16, tag="pcat")
            nc.scalar.dma_start(out=pcat_sb[:], in_=pcat[:])
            fcat_sb = cp.tile([128, FCAT], f32, tag="fcat")
            nc.scalar.dma_start(out=fcat_sb[:], in_=fcat[:])

            def wb_c(c):
                return wcat_sb[:, WB0 + c * HID:WB0 + (c + 1) * HID]

            def ws_c(c):
                return wcat_sb[:, WS0 + c * HID:WS0 + (c + 1) * HID]

            def wa_c(c):
                return wcat_sb[:, WA0 + c * HID:WA0 + (c + 1) * HID]

            def mT_c(c):
                return wcat_sb[:, MT0 + c * R:MT0 + (c + 1) * R]

            wout_sb = wcat_sb[:, WO0:WO0 + 1]
            w1p_sb = pcat_sb[:, PW1P0:PW1P0 + HID]
            b1_sb = fcat_sb[:, FB10:FB10 + 1]
            bout_sb = fcat_sb[:, FBO0:FBO0 + 1]

            # Kick off all gathers as early as possible (desc-gen runs on a
            # Q7 core pair selected by queue_num, so spreading queues lets
            # up to 4 descriptor generations run concurrently).
            bts = []
            for b in range(NB):
                L, o = SIZES[b], OFFS[b]
                bt = btp.tile([128, NCH * L], f16, tag="bt")
                nc.gpsimd.dma_gather(
                    out_ap=bt[:].rearrange("p (c n) -> p c n", c=NCH),
                    in_ap=table[:],
                    idxs_ap=idx_sb[:, o // 16:(o + L) // 16],
                    num_idxs=L,
                    num_idxs_reg=L,
                    elem_size=EMB,
                    transpose=True,
                    queue_num=b % 4,
                )
                bts.append(bt)

            # aT = per-block a^T broadcast (j-repeat of mT) built on-device.
            aT_sb = cp.tile([128, NCH * BLKMAX], f16, tag="aT")
            nc.vector.tensor_copy(
                aT_sb[:].rearrange("p (c j i) -> p c j i", c=NCH, j=BLKMAX // R),
                wcat_sb[:, MT0:MT0 + NCH * R]
                .rearrange("p (c i) -> p c i", c=NCH)[:, :, None, :]
                .broadcast_to([128, NCH, BLKMAX // R, R]),
            )

            scores_sb = cp.tile([128, NPAIR // 128], f32, tag="scores")

            # ma = mentions_shard @ W1a  -> [R, HID]
            ma_ps = psM.tile([R, HID], f32)
            for c in range(NCH):
                nc.tensor.matmul(
                    ma_ps[:],
                    lhsT=mT_c(c),
                    rhs=wa_c(c),
                    start=(c == 0),
                    stop=(c == NCH - 1),
                )
            ma_sb = cp.tile([R, HID], f16, tag="ma")
            nc.scalar.activation(ma_sb[:], ma_ps[:], AF.Copy)

            for b in range(NB):
                L, o = SIZES[b], OFFS[b]
                bt = bts[b]
                st = stp.tile([128, NCH * L], f16, tag="st")
                nc.vector.tensor_mul(
                    st[:].rearrange("p (c n) -> p c n", c=NCH),
                    bt[:].rearrange("p (c n) -> p c n", c=NCH),
                    aT_sb[:].rearrange("p (c n) -> p c n", c=NCH)[:, :, :L],
                )

                hT = psH.tile([128, L], f32, tag="hT")
                nsub = [(0, min(512, L))] + ([(512, L)] if L > 512 else [])
                for lo, hi in nsub:
                    for c in range(NCH):
                        nc.tensor.matmul(
                            hT[:, lo:hi],
                            lhsT=wb_c(c),
                            rhs=bt[:, c * L + lo:c * L + hi],
                            start=(c == 0),
                            stop=False,
                        )
                    for c in range(NCH):
                        nc.tensor.matmul(
                            hT[:, lo:hi],
                            lhsT=ws_c(c),
                            rhs=st[:, c * L + lo:c * L + hi],
                            start=False,
                            stop=False,
                        )
                    nc.tensor.matmul(
                        hT[:, lo:hi],
                        lhsT=w1p_sb,
                        rhs=pcat_sb[:, PPW0 + o + lo:PPW0 + o + hi],
                        start=False,
                        stop=False,
                    )
                    nc.tensor.matmul(
                        hT[:, lo:hi],
                        lhsT=ma_sb[:],
                        rhs=pcat_sb[:, PE0 + o + lo:PE0 + o + hi],
                        start=False,
                        stop=True,
                    )

                hr = hrp.tile([128, L], f16, tag="hr")
                nc.scalar.activation(
                    hr[:], hT[:], AF.Lrelu, bias=b1_sb, scale=1.0, alpha=LEAKY
                )

                ng = L // 128
                sc = psS.tile([128, ng], f32, tag="sc")
                for g in range(ng):
                    nc.tensor.matmul(
                        sc[:, g:g + 1],
                        lhsT=hr[:, g * 128:(g + 1) * 128],
                        rhs=wout_sb,
                        start=True,
                        stop=True,
                    )
                tmp = smp.tile([128, ng], f32, tag="tmp")
                nc.scalar.activation(
                    tmp[:], sc[:], AF.Identity, bias=bout_sb, scale=1.0
                )
                nc.vector.tensor_add(
                    scores_sb[:, o // 128:(o + L) // 128],
                    tmp[:],
                    fcat_sb[:, FRG0 + o // 128:FRG0 + (o + L) // 128],
                )

            nc.sync.dma_start(out=out[:], in_=scores_sb[:])

    nc.compile()
    _CACHE["nc"] = nc
    return nc


def _chunkT(w):
    # [1024, 128] -> [128, 8*128] fp16: column c*128+h holds W[c*128+r, h]
    return np.ascontiguousarray(
        w.reshape(NCH, 128, HID).transpose(1, 0, 2).reshape(128, NCH * HID)
    ).astype(np.float16)


def _host_shared(inputs):
    table = np.asarray(inputs["all_mentions"], np.float32).astype(np.float16)
    W1 = np.asarray(inputs["W1"], np.float32)
    w1a, w1b, w1s, w1p = W1[:1024], W1[1024:2048], W1[2048:3072], W1[3072:]
    return {
        "table": np.ascontiguousarray(table),
        "_wb": _chunkT(w1b),
        "_ws": _chunkT(w1s),
        "_wa": _chunkT(w1a),
        "_w1p": np.ascontiguousarray(w1p).astype(np.float16),
        "_e64": np.ascontiguousarray(
            np.tile(np.eye(R, dtype=np.float16), (1, N_ANTS))
        ),
        "_wout": np.asarray(inputs["W_out"], np.float32).astype(np.float16),
        "_b1c": np.asarray(inputs["b1"], np.float32).reshape(HID, 1),
        "_boutc": np.full(
            (128, 1), np.asarray(inputs["b_out"], np.float32).reshape(())
        ),
    }


def _host_core(inputs, shared, c):
    sl = slice(c * R, (c + 1) * R)
    m = np.asarray(inputs["mentions_batch"], np.float32)[sl]          # [64, 1024]
    pw = np.asarray(inputs["pw_batch"], np.float32)[sl]               # [64, 50, 64]
    idx = np.asarray(inputs["top_indices_batch"])[sl].astype(np.int64)
    rough = np.asarray(inputs["top_rough_scores_batch"], np.float32)[sl]

    idx_perm = idx.T.reshape(NPAIR).astype(np.int16)                  # p' = j*R + i
    idx16 = np.concatenate(
        [
            np.tile(
                idx_perm[OFFS[b]:OFFS[b + 1]].reshape(SIZES[b] // 16, 16).T,
                (8, 1),
            )
            for b in range(NB)
        ],
        axis=1,
    )                                                                 # [128, 200]

    mT = m.reshape(R, NCH, 128).transpose(2, 1, 0).reshape(128, NCH * R)
    pwT = pw.transpose(1, 0, 2).reshape(NPAIR, PW).T                  # [64, 3200]
    rough_pp = rough.T.reshape(NPAIR).reshape(NPAIR // 128, 128).T    # [128, 25]

    wcat = np.concatenate(
        [shared["_wb"], shared["_ws"], shared["_wa"], mT.astype(np.float16),
         shared["_wout"]],
        axis=1,
    )
    pcat = np.concatenate(
        [pwT.astype(np.float16), shared["_e64"], shared["_w1p"]], axis=1
    )
    fcat = np.concatenate(
        [shared["_b1c"], shared["_boutc"], rough_pp], axis=1
    ).astype(np.float32)

    return {
        "idx": np.ascontiguousarray(idx16),
        "wcat": np.ascontiguousarray(wcat),
        "pcat": np.ascontiguousarray(pcat),
        "fcat": np.ascontiguousarray(fcat),
    }


def make_in_maps(inputs):
    shared = _host_shared(inputs)
    table = shared["table"]
    return [
        {"table": table, **_host_core(inputs, shared, c)} for c in range(N_CORES)
    ]


def assemble_output(inputs, results):
    """results: list of per-core dicts with 'out' [128, 25] -> [512, 51] f32."""
    scores = np.empty((BATCH, N_ANTS), np.float32)
    for c in range(N_CORES):
        out_flat = np.asarray(results[c]["out"], np.float32).T.reshape(NPAIR)
        scores[c * R:(c + 1) * R] = out_flat.reshape(N_ANTS, R).T
    dummy = np.full((BATCH, 1), EPS, np.float32)
    return np.concatenate([dummy, scores], axis=1)


def run(inputs, trace=False, **kwargs):
    """Compile (cached), run on 8 cores, return (output, BassKernelResults)."""
    from concourse.bass_utils import run_bass_kernel_spmd

    nc = _build()
    in_maps = make_in_maps(inputs)
    res = run_bass_kernel_spmd(
        nc, in_maps, core_ids=list(range(N_CORES)), trace=trace, **kwargs
    )
    return assemble_output(inputs, res.results), res


def kernel(**inputs) -> np.ndarray:
    out, _ = run(inputs, trace=False)
    return out


# revision 19
# speedup vs baseline: 1.0900x; 1.0149x over previous
"""Trainium2 Bass kernel: AnaphoricityScorer (wl-coref pair FFNN scorer).

Data-parallel over the 512-row mention batch across 8 NeuronCores (64 rows
per core).  Per core (3200 pairs):

  1. The gather b = all_mentions[top_indices] is done with the GPSIMD
     dma_gather(transpose=True) custom DMA, which lands the gathered fp16
     embeddings TRANSPOSED in SBUF: out[p, c, n] = table[idx_n, 128c+p].
     That puts the contraction dim (embedding) on partitions, which is
     exactly the layout the TensorEngine needs for the moving operand.
  2. s = a * b (similarity) is one DVE multiply per block against a
     pre-broadcast a^T tile (built on-device from mT with one DVE copy).
  3. hT[h, pair] = W1b^T b + W1s^T s + W1p^T pw + (a@W1a broadcast) is
     accumulated in PSUM via fp16 matmuls with the W1 chunks as stationary
     operands (the a-term enters via a one-hot moving operand against the
     on-device-computed ma = mentions@W1a).
  4. One ScalarEngine activation applies  leaky_relu(hT + b1)  straight out
     of PSUM into SBUF (fp16).
  5. Layer 2 uses hrelu slices as the stationary operand: out[pair, 1] =
     hrelu_slice^T @ W_out; + b_out (activation bias) + rough (DVE add).

Pair order is "antecedent-major" (p' = j*64 + i) so the a-broadcast is a
clean 64-wide repeat; all permutation/layout work is host-side.

Block sizes are uneven: a small first block shortens the pipeline lead-in
(descriptor generation is ~9ns/desc on the Q7), a small last block
shortens the tail.  Gathers spread across the 4 SWDGE queues so their
descriptor generations run on different Q7 core pairs concurrently.
Constant inputs ride in 3 packed DMAs to keep descriptor count low.
"""

import numpy as np

N_MENTIONS = 10000
BATCH = 512
N_ANTS = 50
EMB = 1024
PW = 64
HID = 128
N_CORES = 8
R = BATCH // N_CORES            # 64 rows per core
NPAIR = R * N_ANTS              # 3200 pairs per core
NCH = EMB // 128                # 8 embedding chunks
EPS = 1e-7
LEAKY = 0.01
N_WARM = 80                     # PE warm-up matmuls (HAM unthrottle)

SIZES = [128, 640, 640, 640, 640, 512]      # pairs per pipeline block
OFFS = np.cumsum([0] + SIZES).tolist()
NB = len(SIZES)
BLKMAX = max(SIZES)
assert OFFS[-1] == NPAIR and all(s % 128 == 0 for s in SIZES)

# wcat column layout (fp16, 128 partitions)
WB0, WS0, WA0 = 0, NCH * HID, 2 * NCH * HID
MT0 = 3 * NCH * HID
WO0 = MT0 + NCH * R
WCAT = WO0 + 1
# pcat column layout (fp16, 64 partitions)
PPW0, PE0, PW1P0 = 0, NPAIR, 2 * NPAIR
PCAT = 2 * NPAIR + HID
# fcat column layout (fp32, 128 partitions)
FB10, FBO0, FRG0 = 0, 1, 2
FCAT = 2 + NPAIR // 128

_CACHE = {}


def _build():
    """Build + compile the (SPMD, per-core identical) Bass program."""
    if "nc" in _CACHE:
        return _CACHE["nc"]
    from concourse import bacc, mybir
    import concourse.tile as tile

    f16, f32, i16 = mybir.dt.float16, mybir.dt.float32, mybir.dt.int16
    AF = mybir.ActivationFunctionType
    nc = bacc.Bacc(num_swdge_queues=4)

    def inp(name, shape, dtype):
        return nc.declare_dram_parameter(name, list(shape), dtype, isOutput=False)

    table = inp("table", [N_MENTIONS, EMB], f16)
    idx = inp("idx", [128, NPAIR // 16], i16)
    wcat = inp("wcat", [128, WCAT], f16)
    pcat = inp("pcat", [PW, PCAT], f16)
    fcat = inp("fcat", [128, FCAT], f32)
    out = nc.declare_dram_parameter("out", [128, NPAIR // 128], f32, isOutput=True)

    with tile.TileContext(nc) as tc:
        with (
            tc.tile_pool(name="const", bufs=1) as cp,
            tc.tile_pool(name="bt", bufs=NB) as btp,
            tc.tile_pool(name="st", bufs=3) as stp,
            tc.tile_pool(name="hr", bufs=2) as hrp,
            tc.tile_pool(name="sm", bufs=2) as smp,
            tc.tile_pool(name="psH", bufs=2, space="PSUM") as psH,
            tc.tile_pool(name="psS", bufs=2, space="PSUM") as psS,
            tc.tile_pool(name="psM", bufs=1, space="PSUM") as psM,
        ):
            # --- warm-up: PE spin + dma_gather first-use probe -----------
            # The PE HAM clock gate needs ~3.4us of sustained matmul work to
            # unthrottle 1.2 -> 2.4 GHz; dependency-free dummy matmuls keep
            # it busy while inputs load, so the real matmuls run warm.
            warm_a = cp.tile([128, 128], f16, tag="warm_a")
            warm_b = cp.tile([128, 512], f16, tag="warm_b")
            nc.gpsimd.memset(warm_a[:], 0)
            nc.gpsimd.memset(warm_b[:], 0)
            warm_ps = psM.tile([128, 512], f32, tag="warm_ps")
            for _ in range(N_WARM):
                nc.tensor.matmul(
                    warm_ps[:], lhsT=warm_a[:], rhs=warm_b[:], start=True, stop=True
                )
            # The first dma_gather execution pays a ~10us one-time cost
            # (ucode library page-in); a dependency-free dummy gather (all
            # indices 0 via memset) absorbs it while inputs stream in.
            scrap_idx = cp.tile([128, 8], i16, tag="scrap_idx")
            nc.gpsimd.memset(scrap_idx[:], 0)
            scrap_bt = cp.tile([128, NCH * 128], f16, tag="scrap_bt")
            nc.gpsimd.dma_gather(
                out_ap=scrap_bt[:].rearrange("p (c n) -> p c n", c=NCH),
                in_ap=table[:],
                idxs_ap=scrap_idx[:],
                num_idxs=128,
                num_idxs_reg=128,
                elem_size=EMB,
                transpose=True,
                queue_num=3,
            )

            idx_sb = cp.tile([128, NPAIR // 16], i16, tag="idx")
            nc.sync.dma_start(out=idx_sb[:], in_=idx[:])

            wcat_sb = cp.tile([128, WCAT], f16, tag="wcat")
            nc.scalar.dma_start(out=wcat_sb[:], in_=wcat[:])
            pcat_sb = cp.tile([PW, PCAT], f16, tag="pcat")
            nc.scalar.dma_start(out=pcat_sb[:], in_=pcat[:])
            fcat_sb = cp.tile([128, FCAT], f32, tag="fcat")
            nc.scalar.dma_start(out=fcat_sb[:], in_=fcat[:])

            def wb_c(c):
                return wcat_sb[:, WB0 + c * HID:WB0 + (c + 1) * HID]

            def ws_c(c):
                return wcat_sb[:, WS0 + c * HID:WS0 + (c + 1) * HID]

            def wa_c(c):
                return wcat_sb[:, WA0 + c * HID:WA0 + (c + 1) * HID]

            def mT_c(c):
                return wcat_sb[:, MT0 + c * R:MT0 + (c + 1) * R]

            wout_sb = wcat_sb[:, WO0:WO0 + 1]
            w1p_sb = pcat_sb[:, PW1P0:PW1P0 + HID]
            b1_sb = fcat_sb[:, FB10:FB10 + 1]
            bout_sb = fcat_sb[:, FBO0:FBO0 + 1]

            # Kick off all gathers as early as possible (desc-gen runs on a
            # Q7 core pair selected by queue_num, so spreading queues lets
            # up to 4 descriptor generations run concurrently).
            bts = []
            for b in range(NB):
                L, o = SIZES[b], OFFS[b]
                bt = btp.tile([128, NCH * L], f16, tag="bt")
                nc.gpsimd.dma_gather(
                    out_ap=bt[:].rearrange("p (c n) -> p c n", c=NCH),
                    in_ap=table[:],
                    idxs_ap=idx_sb[:, o // 16:(o + L) // 16],
                    num_idxs=L,
                    num_idxs_reg=L,
                    elem_size=EMB,
                    transpose=True,
                    queue_num=b % 4,
                )
                bts.append(bt)

            # aT = per-block a^T broadcast (j-repeat of mT) built on-device.
            aT_sb = cp.tile([128, NCH * BLKMAX], f16, tag="aT")
            nc.vector.tensor_copy(
                aT_sb[:].rearrange("p (c j i) -> p c j i", c=NCH, j=BLKMAX // R),
                wcat_sb[:, MT0:MT0 + NCH * R]
                .rearrange("p (c i) -> p c i", c=NCH)[:, :, None, :]
                .broadcast_to([128, NCH, BLKMAX // R, R]),
            )

            scores_sb = cp.tile([128, NPAIR // 128], f32, tag="scores")

            # ma = mentions_shard @ W1a  -> [R, HID]
            ma_ps = psM.tile([R, HID], f32)
            for c in range(NCH):
                nc.tensor.matmul(
                    ma_ps[:],
                    lhsT=mT_c(c),
                    rhs=wa_c(c),
                    start=(c == 0),
                    stop=(c == NCH - 1),
                )
            ma_sb = cp.tile([R, HID], f16, tag="ma")
            nc.scalar.activation(ma_sb[:], ma_ps[:], AF.Copy)

            for b in range(NB):
                L, o = SIZES[b], OFFS[b]
                bt = bts[b]
                st = stp.tile([128, NCH * L], f16, tag="st")
                # Two half-chunk TT ops so the S-matmuls of the first half
                # overlap the DVE multiply of the second half.
                for h in range(2):
                    cs = slice(h * (NCH // 2), (h + 1) * (NCH // 2))
                    nc.vector.tensor_mul(
                        st[:].rearrange("p (c n) -> p c n", c=NCH)[:, cs],
                        bt[:].rearrange("p (c n) -> p c n", c=NCH)[:, cs],
                        aT_sb[:].rearrange("p (c n) -> p c n", c=NCH)[:, cs, :L],
                    )

                hT = psH.tile([128, L], f32, tag="hT")
                nsub = [(0, min(512, L))] + ([(512, L)] if L > 512 else [])
                for lo, hi in nsub:
                    for c in range(NCH):
                        nc.tensor.matmul(
                            hT[:, lo:hi],
                            lhsT=wb_c(c),
                            rhs=bt[:, c * L + lo:c * L + hi],
                            start=(c == 0),
                            stop=False,
                        )
                    for c in range(NCH):
                        nc.tensor.matmul(
                            hT[:, lo:hi],
                            lhsT=ws_c(c),
                            rhs=st[:, c * L + lo:c * L + hi],
                            start=False,
                            stop=False,
                        )
                    nc.tensor.matmul(
                        hT[:, lo:hi],
                        lhsT=w1p_sb,
                        rhs=pcat_sb[:, PPW0 + o + lo:PPW0 + o + hi],
                        start=False,
                        stop=False,
                    )
                    nc.tensor.matmul(
                        hT[:, lo:hi],
                        lhsT=ma_sb[:],
                        rhs=pcat_sb[:, PE0 + o + lo:PE0 + o + hi],
                        start=False,
                        stop=True,
                    )

                hr = hrp.tile([128, L], f16, tag="hr")
                nc.scalar.activation(
                    hr[:], hT[:], AF.Lrelu, bias=b1_sb, scale=1.0, alpha=LEAKY
                )

                ng = L // 128
                sc = psS.tile([128, ng], f32, tag="sc")
                for g in range(ng):
                    nc.tensor.matmul(
                        sc[:, g:g + 1],
                        lhsT=hr[:, g * 128:(g + 1) * 128],
                        rhs=wout_sb,
                        start=True,
                        stop=True,
                    )
                tmp = smp.tile([128, ng], f32, tag="tmp")
                nc.scalar.activation(
                    tmp[:], sc[:], AF.Identity, bias=bout_sb, scale=1.0
                )
                nc.vector.tensor_add(
                    scores_sb[:, o // 128:(o + L) // 128],
                    tmp[:],
                    fcat_sb[:, FRG0 + o // 128:FRG0 + (o + L) // 128],
                )

            nc.sync.dma_start(out=out[:], in_=scores_sb[:])

    nc.compile()
    _CACHE["nc"] = nc
    return nc


def _chunkT(w):
    # [1024, 128] -> [128, 8*128] fp16: column c*128+h holds W[c*128+r, h]
    return np.ascontiguousarray(
        w.reshape(NCH, 128, HID).transpose(1, 0, 2).reshape(128, NCH * HID)
    ).astype(np.float16)


def _host_shared(inputs):
    table = np.asarray(inputs["all_mentions"], np.float32).astype(np.float16)
    W1 = np.asarray(inputs["W1"], np.float32)
    w1a, w1b, w1s, w1p = W1[:1024], W1[1024:2048], W1[2048:3072], W1[3072:]
    return {
        "table": np.ascontiguousarray(table),
        "_wb": _chunkT(w1b),
        "_ws": _chunkT(w1s),
        "_wa": _chunkT(w1a),
        "_w1p": np.ascontiguousarray(w1p).astype(np.float16),
        "_e64": np.ascontiguousarray(
            np.tile(np.eye(R, dtype=np.float16), (1, N_ANTS))
        ),
        "_wout": np.asarray(inputs["W_out"], np.float32).astype(np.float16),
        "_b1c": np.asarray(inputs["b1"], np.float32).reshape(HID, 1),
        "_boutc": np.full(
            (128, 1), np.asarray(inputs["b_out"], np.float32).reshape(())
        ),
    }


def _host_core(inputs, shared, c):
    sl = slice(c * R, (c + 1) * R)
    m = np.asarray(inputs["mentions_batch"], np.float32)[sl]          # [64, 1024]
    pw = np.asarray(inputs["pw_batch"], np.float32)[sl]               # [64, 50, 64]
    idx = np.asarray(inputs["top_indices_batch"])[sl].astype(np.int64)
    rough = np.asarray(inputs["top_rough_scores_batch"], np.float32)[sl]

    idx_perm = idx.T.reshape(NPAIR).astype(np.int16)                  # p' = j*R + i
    idx16 = np.concatenate(
        [
            np.tile(
                idx_perm[OFFS[b]:OFFS[b + 1]].reshape(SIZES[b] // 16, 16).T,
                (8, 1),
            )
            for b in range(NB)
        ],
        axis=1,
    )                                                                 # [128, 200]

    mT = m.reshape(R, NCH, 128).transpose(2, 1, 0).reshape(128, NCH * R)
    pwT = pw.transpose(1, 0, 2).reshape(NPAIR, PW).T                  # [64, 3200]
    rough_pp = rough.T.reshape(NPAIR).reshape(NPAIR // 128, 128).T    # [128, 25]

    wcat = np.concatenate(
        [shared["_wb"], shared["_ws"], shared["_wa"], mT.astype(np.float16),
         shared["_wout"]],
        axis=1,
    )
    pcat = np.concatenate(
        [pwT.astype(np.float16), shared["_e64"], shared["_w1p"]], axis=1
    )
    fcat = np.concatenate(
        [shared["_b1c"], shared["_boutc"], rough_pp], axis=1
    ).astype(np.float32)

    return {
        "idx": np.ascontiguousarray(idx16),
        "wcat": np.ascontiguousarray(wcat),
        "pcat": np.ascontiguousarray(pcat),
        "fcat": np.ascontiguousarray(fcat),
    }


def make_in_maps(inputs):
    shared = _host_shared(inputs)
    table = shared["table"]
    return [
        {"table": table, **_host_core(inputs, shared, c)} for c in range(N_CORES)
    ]


def assemble_output(inputs, results):
    """results: list of per-core dicts with 'out' [128, 25] -> [512, 51] f32."""
    scores = np.empty((BATCH, N_ANTS), np.float32)
    for c in range(N_CORES):
        out_flat = np.asarray(results[c]["out"], np.float32).T.reshape(NPAIR)
        scores[c * R:(c + 1) * R] = out_flat.reshape(N_ANTS, R).T
    dummy = np.full((BATCH, 1), EPS, np.float32)
    return np.concatenate([dummy, scores], axis=1)


def run(inputs, trace=False, **kwargs):
    """Compile (cached), run on 8 cores, return (output, BassKernelResults)."""
    from concourse.bass_utils import run_bass_kernel_spmd

    nc = _build()
    in_maps = make_in_maps(inputs)
    res = run_bass_kernel_spmd(
        nc, in_maps, core_ids=list(range(N_CORES)), trace=trace, **kwargs
    )
    return assemble_output(inputs, res.results), res


def kernel(**inputs) -> np.ndarray:
    out, _ = run(inputs, trace=False)
    return out


# revision 23
# speedup vs baseline: 1.1363x; 1.0425x over previous
"""Trainium2 Bass kernel: AnaphoricityScorer (wl-coref pair FFNN scorer).

Data-parallel over the 512-row mention batch across 8 NeuronCores (64 rows
per core).  Per core (3200 pairs):

  1. The gather b = all_mentions[top_indices] is done with the GPSIMD
     dma_gather(transpose=True) custom DMA, which lands the gathered fp16
     embeddings TRANSPOSED in SBUF: out[p, c, n] = table[idx_n, 128c+p].
     That puts the contraction dim (embedding) on partitions, which is
     exactly the layout the TensorEngine needs for the moving operand.
  2. s = a * b (similarity) is one DVE multiply per block against a
     pre-broadcast a^T tile (built on-device from mT with one DVE copy).
  3. hT[h, pair] = W1b^T b + W1s^T s + W1p^T pw + (a@W1a broadcast) is
     accumulated in PSUM via fp16 matmuls with the W1 chunks as stationary
     operands (the a-term enters via a one-hot moving operand against the
     on-device-computed ma = mentions@W1a).
  4. One ScalarEngine activation applies  leaky_relu(hT + b1)  straight out
     of PSUM into SBUF (fp16).
  5. Layer 2 uses hrelu slices as the stationary operand: out[pair, 1] =
     hrelu_slice^T @ W_out; + b_out (activation bias) + rough (DVE add).

Pair order is "antecedent-major" (p' = j*64 + i) so the a-broadcast is a
clean 64-wide repeat; all permutation/layout work is host-side.

Block sizes are uneven: a small first block shortens the pipeline lead-in
(descriptor generation is ~9ns/desc on the Q7), a small last block
shortens the tail.  Gathers spread across the 4 SWDGE queues so their
descriptor generations run on different Q7 core pairs concurrently.
Constant inputs ride in 3 packed DMAs to keep descriptor count low.
"""

import numpy as np

N_MENTIONS = 10000
BATCH = 512
N_ANTS = 50
EMB = 1024
PW = 64
HID = 128
N_CORES = 8
R = BATCH // N_CORES            # 64 rows per core
NPAIR = R * N_ANTS              # 3200 pairs per core
NCH = EMB // 128                # 8 embedding chunks
EPS = 1e-7
LEAKY = 0.01
N_WARM = 65                     # PE warm-up matmuls (HAM unthrottle)

SIZES = [128, 512, 512, 512, 512, 512, 512]  # pairs per pipeline block
OFFS = np.cumsum([0] + SIZES).tolist()
NB = len(SIZES)
BLKMAX = max(SIZES)
assert OFFS[-1] == NPAIR and all(s % 128 == 0 for s in SIZES)

# wcat column layout (fp16, 128 partitions)
WB0, WS0, WA0 = 0, NCH * HID, 2 * NCH * HID
MT0 = 3 * NCH * HID
WO0 = MT0 + NCH * R
WCAT = WO0 + 1
# pcat column layout (fp16, 64 partitions)
PPW0, PE0, PW1P0 = 0, NPAIR, 2 * NPAIR
PCAT = 2 * NPAIR + HID
# fcat column layout (fp32, 128 partitions)
FB10, FBO0, FRG0 = 0, 1, 2
FCAT = 2 + NPAIR // 128

_CACHE = {}


def _build():
    """Build + compile the (SPMD, per-core identical) Bass program."""
    if "nc" in _CACHE:
        return _CACHE["nc"]
    from concourse import bacc, mybir
    import concourse.tile as tile

    f16, f32, i16 = mybir.dt.float16, mybir.dt.float32, mybir.dt.int16
    AF = mybir.ActivationFunctionType
    nc = bacc.Bacc(num_swdge_queues=4)

    def inp(name, shape, dtype):
        return nc.declare_dram_parameter(name, list(shape), dtype, isOutput=False)

    table = inp("table", [N_MENTIONS, EMB], f16)
    idx = inp("idx", [128, NPAIR // 16], i16)
    wcat = inp("wcat", [128, WCAT], f16)
    pcat = inp("pcat", [PW, PCAT], f16)
    fcat = inp("fcat", [128, FCAT], f32)
    out = nc.declare_dram_parameter("out", [128, NPAIR // 128], f32, isOutput=True)

    with tile.TileContext(nc) as tc:
        with (
            tc.tile_pool(name="const", bufs=1) as cp,
            tc.tile_pool(name="bt", bufs=NB) as btp,
            tc.tile_pool(name="st", bufs=3) as stp,
            tc.tile_pool(name="hr", bufs=2) as hrp,
            tc.tile_pool(name="sm", bufs=2) as smp,
            tc.tile_pool(name="psH", bufs=3, space="PSUM") as psH,
            tc.tile_pool(name="psS", bufs=2, space="PSUM") as psS,
            tc.tile_pool(name="psM", bufs=1, space="PSUM") as psM,
        ):
            # --- warm-up: PE spin + dma_gather first-use probe -----------
            # The PE HAM clock gate needs ~3.4us of sustained matmul work to
            # unthrottle 1.2 -> 2.4 GHz; dependency-free dummy matmuls keep
            # it busy while inputs load, so the real matmuls run warm.
            warm_a = cp.tile([128, 128], f16, tag="warm_a")
            warm_b = cp.tile([128, 512], f16, tag="warm_b")
            nc.gpsimd.memset(warm_a[:], 0)
            nc.gpsimd.memset(warm_b[:], 0)
            warm_ps = psM.tile([128, 512], f32, tag="warm_ps")
            for _ in range(N_WARM):
                nc.tensor.matmul(
                    warm_ps[:], lhsT=warm_a[:], rhs=warm_b[:], start=True, stop=True
                )
            # The first dma_gather execution pays a ~10us one-time cost
            # (ucode library page-in); a dependency-free dummy gather (all
            # indices 0 via memset) absorbs it while inputs stream in.
            scrap_idx = cp.tile([128, 8], i16, tag="scrap_idx")
            nc.gpsimd.memset(scrap_idx[:], 0)
            scrap_bt = cp.tile([128, NCH * 128], f16, tag="scrap_bt")
            nc.gpsimd.dma_gather(
                out_ap=scrap_bt[:].rearrange("p (c n) -> p c n", c=NCH),
                in_ap=table[:],
                idxs_ap=scrap_idx[:],
                num_idxs=128,
                num_idxs_reg=128,
                elem_size=EMB,
                transpose=True,
                queue_num=3,
            )

            idx_sb = cp.tile([128, NPAIR // 16], i16, tag="idx")
            nc.sync.dma_start(out=idx_sb[:], in_=idx[:])

            wcat_sb = cp.tile([128, WCAT], f16, tag="wcat")
            nc.scalar.dma_start(out=wcat_sb[:], in_=wcat[:])
            pcat_sb = cp.tile([PW, PCAT], f16, tag="pcat")
            nc.scalar.dma_start(out=pcat_sb[:], in_=pcat[:])
            fcat_sb = cp.tile([128, FCAT], f32, tag="fcat")
            nc.scalar.dma_start(out=fcat_sb[:], in_=fcat[:])

            def wb_c(c):
                return wcat_sb[:, WB0 + c * HID:WB0 + (c + 1) * HID]

            def ws_c(c):
                return wcat_sb[:, WS0 + c * HID:WS0 + (c + 1) * HID]

            def wa_c(c):
                return wcat_sb[:, WA0 + c * HID:WA0 + (c + 1) * HID]

            def mT_c(c):
                return wcat_sb[:, MT0 + c * R:MT0 + (c + 1) * R]

            wout_sb = wcat_sb[:, WO0:WO0 + 1]
            w1p_sb = pcat_sb[:, PW1P0:PW1P0 + HID]
            b1_sb = fcat_sb[:, FB10:FB10 + 1]
            bout_sb = fcat_sb[:, FBO0:FBO0 + 1]

            # Kick off all gathers as early as possible (desc-gen runs on a
            # Q7 core pair selected by queue_num, so spreading queues lets
            # up to 4 descriptor generations run concurrently).
            bts = []
            for b in range(NB):
                L, o = SIZES[b], OFFS[b]
                bt = btp.tile([128, NCH * L], f16, tag="bt")
                nc.gpsimd.dma_gather(
                    out_ap=bt[:].rearrange("p (c n) -> p c n", c=NCH),
                    in_ap=table[:],
                    idxs_ap=idx_sb[:, o // 16:(o + L) // 16],
                    num_idxs=L,
                    num_idxs_reg=L,
                    elem_size=EMB,
                    transpose=True,
                    queue_num=b % 4,
                )
                bts.append(bt)

            # aT = per-block a^T broadcast (j-repeat of mT) built on-device.
            aT_sb = cp.tile([128, NCH * BLKMAX], f16, tag="aT")
            nc.vector.tensor_copy(
                aT_sb[:].rearrange("p (c j i) -> p c j i", c=NCH, j=BLKMAX // R),
                wcat_sb[:, MT0:MT0 + NCH * R]
                .rearrange("p (c i) -> p c i", c=NCH)[:, :, None, :]
                .broadcast_to([128, NCH, BLKMAX // R, R]),
            )

            scores_sb = cp.tile([128, NPAIR // 128], f32, tag="scores")

            # ma = mentions_shard @ W1a  -> [R, HID]
            ma_ps = psM.tile([R, HID], f32)
            for c in range(NCH):
                nc.tensor.matmul(
                    ma_ps[:],
                    lhsT=mT_c(c),
                    rhs=wa_c(c),
                    start=(c == 0),
                    stop=(c == NCH - 1),
                )
            ma_sb = cp.tile([R, HID], f16, tag="ma")
            nc.scalar.activation(ma_sb[:], ma_ps[:], AF.Copy)

            for b in range(NB):
                L, o = SIZES[b], OFFS[b]
                bt = bts[b]
                st = stp.tile([128, NCH * L], f16, tag="st")
                # Two half-chunk TT ops so the S-matmuls of the first half
                # overlap the DVE multiply of the second half.
                for h in range(2):
                    cs = slice(h * (NCH // 2), (h + 1) * (NCH // 2))
                    nc.vector.tensor_mul(
                        st[:].rearrange("p (c n) -> p c n", c=NCH)[:, cs],
                        bt[:].rearrange("p (c n) -> p c n", c=NCH)[:, cs],
                        aT_sb[:].rearrange("p (c n) -> p c n", c=NCH)[:, cs, :L],
                    )

                hT = psH.tile([128, L], f32, tag="hT")
                nsub = [(0, min(512, L))] + ([(512, L)] if L > 512 else [])
                for lo, hi in nsub:
                    # pw/a terms first: they have no gather dependency, so
                    # the PE can run them ahead while transfers stream.
                    nc.tensor.matmul(
                        hT[:, lo:hi],
                        lhsT=w1p_sb,
                        rhs=pcat_sb[:, PPW0 + o + lo:PPW0 + o + hi],
                        start=True,
                        stop=False,
                    )
                    nc.tensor.matmul(
                        hT[:, lo:hi],
                        lhsT=ma_sb[:],
                        rhs=pcat_sb[:, PE0 + o + lo:PE0 + o + hi],
                        start=False,
                        stop=False,
                    )
                    for c in range(NCH):
                        nc.tensor.matmul(
                            hT[:, lo:hi],
                            lhsT=wb_c(c),
                            rhs=bt[:, c * L + lo:c * L + hi],
                            start=False,
                            stop=False,
                        )
                    for c in range(NCH):
                        nc.tensor.matmul(
                            hT[:, lo:hi],
                            lhsT=ws_c(c),
                            rhs=st[:, c * L + lo:c * L + hi],
                            start=False,
                            stop=(c == NCH - 1),
                        )

                hr = hrp.tile([128, L], f16, tag="hr")
                nc.scalar.activation(
                    hr[:], hT[:], AF.Lrelu, bias=b1_sb, scale=1.0, alpha=LEAKY
                )

                ng = L // 128
                sc = psS.tile([128, ng], f32, tag="sc")
                for g in range(ng):
                    nc.tensor.matmul(
                        sc[:, g:g + 1],
                        lhsT=hr[:, g * 128:(g + 1) * 128],
                        rhs=wout_sb,
                        start=True,
                        stop=True,
                    )
                tmp = smp.tile([128, ng], f32, tag="tmp")
                nc.scalar.activation(
                    tmp[:], sc[:], AF.Identity, bias=bout_sb, scale=1.0
                )
                nc.vector.tensor_add(
                    scores_sb[:, o // 128:(o + L) // 128],
                    tmp[:],
                    fcat_sb[:, FRG0 + o // 128:FRG0 + (o + L) // 128],
                )

            nc.sync.dma_start(out=out[:], in_=scores_sb[:])

    nc.compile()
    _CACHE["nc"] = nc
    return nc


def _chunkT(w):
    # [1024, 128] -> [128, 8*128] fp16: column c*128+h holds W[c*128+r, h]
    return np.ascontiguousarray(
        w.reshape(NCH, 128, HID).transpose(1, 0, 2).reshape(128, NCH * HID)
    ).astype(np.float16)


def _host_shared(inputs):
    table = np.asarray(inputs["all_mentions"], np.float32).astype(np.float16)
    W1 = np.asarray(inputs["W1"], np.float32)
    w1a, w1b, w1s, w1p = W1[:1024], W1[1024:2048], W1[2048:3072], W1[3072:]
    return {
        "table": np.ascontiguousarray(table),
        "_wb": _chunkT(w1b),
        "_ws": _chunkT(w1s),
        "_wa": _chunkT(w1a),
        "_w1p": np.ascontiguousarray(w1p).astype(np.float16),
        "_e64": np.ascontiguousarray(
            np.tile(np.eye(R, dtype=np.float16), (1, N_ANTS))
        ),
        "_wout": np.asarray(inputs["W_out"], np.float32).astype(np.float16),
        "_b1c": np.asarray(inputs["b1"], np.float32).reshape(HID, 1),
        "_boutc": np.full(
            (128, 1), np.asarray(inputs["b_out"], np.float32).reshape(())
        ),
    }


def _host_core(inputs, shared, c):
    sl = slice(c * R, (c + 1) * R)
    m = np.asarray(inputs["mentions_batch"], np.float32)[sl]          # [64, 1024]
    pw = np.asarray(inputs["pw_batch"], np.float32)[sl]               # [64, 50, 64]
    idx = np.asarray(inputs["top_indices_batch"])[sl].astype(np.int64)
    rough = np.asarray(inputs["top_rough_scores_batch"], np.float32)[sl]

    idx_perm = idx.T.reshape(NPAIR).astype(np.int16)                  # p' = j*R + i
    idx16 = np.concatenate(
        [
            np.tile(
                idx_perm[OFFS[b]:OFFS[b + 1]].reshape(SIZES[b] // 16, 16).T,
                (8, 1),
            )
            for b in range(NB)
        ],
        axis=1,
    )                                                                 # [128, 200]

    mT = m.reshape(R, NCH, 128).transpose(2, 1, 0).reshape(128, NCH * R)
    pwT = pw.transpose(1, 0, 2).reshape(NPAIR, PW).T                  # [64, 3200]
    rough_pp = rough.T.reshape(NPAIR).reshape(NPAIR // 128, 128).T    # [128, 25]

    wcat = np.concatenate(
        [shared["_wb"], shared["_ws"], shared["_wa"], mT.astype(np.float16),
         shared["_wout"]],
        axis=1,
    )
    pcat = np.concatenate(
        [pwT.astype(np.float16), shared["_e64"], shared["_w1p"]], axis=1
    )
    fcat = np.concatenate(
        [shared["_b1c"], shared["_boutc"], rough_pp], axis=1
    ).astype(np.float32)

    return {
        "idx": np.ascontiguousarray(idx16),
        "wcat": np.ascontiguousarray(wcat),
        "pcat": np.ascontiguousarray(pcat),
        "fcat": np.ascontiguousarray(fcat),
    }


def make_in_maps(inputs):
    shared = _host_shared(inputs)
    table = shared["table"]
    return [
        {"table": table, **_host_core(inputs, shared, c)} for c in range(N_CORES)
    ]


def assemble_output(inputs, results):
    """results: list of per-core dicts with 'out' [128, 25] -> [512, 51] f32."""
    scores = np.empty((BATCH, N_ANTS), np.float32)
    for c in range(N_CORES):
        out_flat = np.asarray(results[c]["out"], np.float32).T.reshape(NPAIR)
        scores[c * R:(c + 1) * R] = out_flat.reshape(N_ANTS, R).T
    dummy = np.full((BATCH, 1), EPS, np.float32)
    return np.concatenate([dummy, scores], axis=1)


def run(inputs, trace=False, **kwargs):
    """Compile (cached), run on 8 cores, return (output, BassKernelResults)."""
    from concourse.bass_utils import run_bass_kernel_spmd

    nc = _build()
    in_maps = make_in_maps(inputs)
    res = run_bass_kernel_spmd(
        nc, in_maps, core_ids=list(range(N_CORES)), trace=trace, **kwargs
    )
    return assemble_output(inputs, res.results), res


def kernel(**inputs) -> np.ndarray:
    out, _ = run(inputs, trace=False)
    return out
